# revision 1
# baseline (speedup 1.0000x reference)
"""Multi-head attention (B=4, S=2048, E=1024, H=16, D=64) on 8 TRN2 cores.

Sharding: core c handles batch b = c//2, query half = c%2 (1024 queries).
Each core computes K/V over its batch's full sequence (duplicated between the
two half-cores of a batch -- cheaper at these sizes than any collective),
attention for all 16 heads over its 1024 queries, and the output projection
for its output chunk. Outputs are disjoint -> host gather is concatenation.

The host rotates each core's sequence so its query block is always rows
0:1024 (attention is permutation-invariant over keys), and pre-transposes the
weights and activations (pure layout prep) so the e-contraction projections
have e on partitions.

Precision: float32r (TF32-like, full PE rate) for all matmuls; exp and
accumulations in fp32.

The emission is software-pipelined: head-pair hp+1's Q/K/V projection
instructions are interleaved into head-pair hp's attention stream so the PE's
in-order queue can fill the gaps left while ACT computes exp().
"""

from contextlib import ExitStack

import numpy as np

import concourse.bass as bass
import concourse.tile as tile
from concourse import bacc, mybir
from concourse.bass_utils import run_bass_kernel_spmd

dt = mybir.dt
AF = mybir.ActivationFunctionType

B, S, E, H, D = 4, 2048, 1024, 16, 64
N_CORES = 8
SQ = 1024          # queries per core
P = 128
EC = E // P        # 8 e-chunks
TC = S // P        # 16 t-chunks (keys)
QC = SQ // P       # 8 query chunks
HP = H // 2        # 8 head-pairs

DEBUG = False
PV_DT = "float32r"  # dtype for the probs/V matmul operands

_SCRATCH_N = [0]


def _emit(nc, tc, xt_d, wqt, wkt, wvt, wot, bo, y, dbg=None):
    f32, f32r = dt.float32, dt.float32r
    pv_dt = getattr(dt, PV_DT)

    _SCRATCH_N[0] += 1
    on_d = nc.dram_tensor(f"on_scratch_{_SCRATCH_N[0]}", [E, SQ],
                          dt.float32).ap()

    with ExitStack() as ctx:
        const = ctx.enter_context(tc.tile_pool(name="const", bufs=1))
        ps = ctx.enter_context(tc.tile_pool(name="ps", bufs=2, space="PSUM"))
        ps_p = ctx.enter_context(
            tc.tile_pool(name="ps_p", bufs=1, space="PSUM"))
        ps_o = ctx.enter_context(
            tc.tile_pool(name="ps_o", bufs=3, space="PSUM"))

        ones_col = const.tile([P, 1], f32)
        nc.vector.memset(ones_col[:], 1.0)

        with ExitStack() as actx:
            xt_pool = actx.enter_context(tc.tile_pool(name="xt", bufs=1))
            w1 = actx.enter_context(tc.tile_pool(name="w1", bufs=1))
            w2 = actx.enter_context(tc.tile_pool(name="w2", bufs=2))
            vp_pool = actx.enter_context(tc.tile_pool(name="vp", bufs=2))
            ut_pool = actx.enter_context(tc.tile_pool(name="ut", bufs=4))

            # ---- phase 0: xT in SBUF (f32r); queries are cols 0:1024 ----
            xT = xt_pool.tile([P, EC, S], f32r)
            xt_view = xt_d.rearrange("(o p) t -> p o t", p=P)
            for tc_i in range(TC):
                x_sb = w2.tile([P, EC, P], f32, tag="xdma")
                nc.sync.dma_start(
                    x_sb[:], xt_view[:, :, tc_i * P:(tc_i + 1) * P])
                nc.vector.tensor_copy(
                    xT[:, :, tc_i * P:(tc_i + 1) * P], x_sb[:])

            if dbg is not None:
                nc.sync.dma_start(dbg["xt"], xT[:].bitcast(f32))

            qt_t, kt_t, vp_t = {}, {}, {}

            def proj_ops(hp):
                ops = []
                st = {}

                def wload():
                    w_sb = w1.tile([P, EC, 2, P], f32, tag="wdma")
                    for wi, w_dram in enumerate((wqt, wkt)):
                        nc.sync.dma_start(
                            w_sb[:, :, wi, :],
                            w_dram.rearrange("(o p) f -> p o f", p=P)[
                                :, :, hp * P:(hp + 1) * P])
                    st["w_r"] = w1.tile([P, EC, 2, P], f32r, tag="wr", name=f"wr{hp}")
                    nc.vector.tensor_copy(st["w_r"][:], w_sb[:])
                ops.append(wload)

                # QT: two q-halves, each accumulated over ec in own psum
                for nq in range(SQ // 512):
                    def qalloc(nq=nq):
                        if nq == 0:
                            qt_t[hp] = w2.tile([P, SQ], f32r, tag="qt", name=f"qt{hp}")
                        st["pq"] = ps_p.tile([P, 512], f32, tag="PROJ", name=f"pq{hp}_{nq}")
                    ops.append(qalloc)
                    for ec in range(EC):
                        def qmm(ec=ec, nq=nq):
                            nc.tensor.matmul(
                                st["pq"][:], st["w_r"][:, ec, 0],
                                xT[:, ec, nq * 512:(nq + 1) * 512],
                                start=(ec == 0), stop=(ec == EC - 1))
                        ops.append(qmm)

                    def qcopy(nq=nq):
                        nc.vector.tensor_copy(
                            qt_t[hp][:, nq * 512:(nq + 1) * 512], st["pq"][:])
                    ops.append(qcopy)

                # KT: four 512-chunks
                for nk in range(S // 512):
                    def kalloc(nk=nk):
                        if nk == 0:
                            kt_t[hp] = w2.tile([P, S], f32r, tag="kt", name=f"kt{hp}")
                        st["pk"] = ps_p.tile([P, 512], f32, tag="PROJ", name=f"pk{hp}_{nk}")
                    ops.append(kalloc)
                    for ec in range(EC):
                        def kmm(ec=ec, nk=nk):
                            nc.tensor.matmul(
                                st["pk"][:], st["w_r"][:, ec, 1],
                                xT[:, ec, nk * 512:(nk + 1) * 512],
                                start=(ec == 0), stop=(ec == EC - 1))
                        ops.append(kmm)

                    def kcopy(nk=nk):
                        nc.vector.tensor_copy(
                            kt_t[hp][:, nk * 512:(nk + 1) * 512], st["pk"][:])
                    ops.append(kcopy)

                # V for the pair (hp, hp+1) on even hp: out free dim 256
                if hp % 2 == 0:
                    def vload():
                        wv_sb = w1.tile([P, EC, 2 * P], f32, tag="wdma_v")
                        nc.sync.dma_start(
                            wv_sb[:],
                            wvt.rearrange("(o p) f -> p o f", p=P)[
                                :, :, hp * P:(hp + 2) * P])
                        st["wv_r"] = w1.tile([P, EC, 2 * P], f32r, tag="wr_v", name=f"wvr{hp}")
                        nc.vector.tensor_copy(st["wv_r"][:], wv_sb[:])
                        vp_t[hp // 2] = vp_pool.tile(
                            [P, TC, 4, 65], pv_dt, tag="vp",
                            name=f"vp{hp // 2}")
                        nc.vector.tensor_copy(
                            vp_t[hp // 2][:, :, :, 64:65],
                            ones_col[:, None, None, :].to_broadcast(
                                [P, TC, 4, 1]))
                    ops.append(vload)
                    for tc_i in range(TC):
                        def valloc(tc_i=tc_i):
                            st["pv"] = ps_p.tile([P, 512], f32, tag="PROJ", name=f"pv{hp}_{tc_i}")
                        ops.append(valloc)
                        for ec in range(EC):
                            def vmm(ec=ec, tc_i=tc_i):
                                nc.tensor.matmul(
                                    st["pv"][:, :256],
                                    xT[:, ec, tc_i * P:(tc_i + 1) * P],
                                    st["wv_r"][:, ec, :],
                                    start=(ec == 0), stop=(ec == EC - 1))
                            ops.append(vmm)

                        def vcopy(tc_i=tc_i):
                            nc.vector.tensor_copy(
                                vp_t[hp // 2][:, tc_i, :, 0:64],
                                st["pv"][:, :256].rearrange(
                                    "p (h d) -> p h d", h=4))
                        ops.append(vcopy)
                return ops

            # prologue: head-pair 0's projections emitted standalone
            for op in proj_ops(0):
                op()

            for hp in range(HP):
                qt, kt = qt_t[hp], kt_t[hp]
                vp = vp_t[hp // 2]
                ha, hb = 2 * (hp % 2), 2 * (hp % 2) + 1
                nxt = proj_ops(hp + 1) if hp + 1 < HP else []
                n_emit = 0
                n_iter = 2 * TC
                it = 0

                for qh in range(2):
                    po_a = ps_o.tile([65, 512], f32, tag="po")
                    po_b = ps_o.tile([65, 512], f32, tag="po")
                    qs = slice(qh * 512, (qh + 1) * 512)
                    for kc in range(TC):
                        sc = ps.tile([P, 1024], f32, tag="S")
                        nc.tensor.matmul(
                            sc[:, 0:512], kt[0:64, kc * P:(kc + 1) * P],
                            qt[0:64, qs], start=True, stop=True)
                        nc.tensor.matmul(
                            sc[:, 512:1024], kt[64:128, kc * P:(kc + 1) * P],
                            qt[64:128, qs], start=True, stop=True)
                        ut = ut_pool.tile([P, 1024], pv_dt, tag="ut")
                        nc.scalar.activation(
                            ut[:], sc[:], AF.Exp, scale=0.125)
                        nc.tensor.matmul(
                            po_a[:], vp[:, kc, ha], ut[:, 0:512],
                            start=(kc == 0), stop=(kc == TC - 1))
                        nc.tensor.matmul(
                            po_b[:], vp[:, kc, hb], ut[:, 512:1024],
                            start=(kc == 0), stop=(kc == TC - 1))
                        # interleave next head-pair's projection work
                        it += 1
                        target = len(nxt) * it // n_iter
                        while n_emit < target:
                            nxt[n_emit]()
                            n_emit += 1

                    # normalize; row 64 of po_x is the softmax denominator.
                    # partition_broadcast only writes correctly from base 0:
                    # broadcast into a full tile, slice at read time.
                    on_sb = w2.tile([P, 512], f32, tag="on_sb")
                    rcp_a = w1.tile([1, 512], f32, tag="rcp_a")
                    nc.vector.reciprocal(rcp_a[:], po_a[64:65, :])
                    brec_a = w1.tile([P, 512], f32, tag="brec_a")
                    nc.gpsimd.partition_broadcast(brec_a[:], rcp_a[:])
                    nc.vector.tensor_mul(
                        on_sb[0:64, :], po_a[0:64, :], brec_a[0:64, :])
                    rcp_b = w1.tile([1, 512], f32, tag="rcp_b")
                    nc.vector.reciprocal(rcp_b[:], po_b[64:65, :])
                    brec_b = w1.tile([P, 512], f32, tag="brec_b")
                    nc.gpsimd.partition_broadcast(brec_b[:], rcp_b[:])
                    nc.vector.tensor_mul(
                        on_sb[64:128, :], po_b[0:64, :], brec_b[64:128, :])
                    nc.sync.dma_start(
                        on_d[hp * P:(hp + 1) * P, qh * 512:(qh + 1) * 512],
                        on_sb[:])
                assert n_emit == len(nxt)

        if dbg is not None:
            nc.sync.dma_start(dbg["on"], on_d[:])

        # ---- output projection: y = ON^T @ WoT + bo ----
        with ExitStack() as dctx:
            wo_pool = dctx.enter_context(tc.tile_pool(name="wo", bufs=2))
            yp = dctx.enter_context(tc.tile_pool(name="yp", bufs=2))

            bo_one = wo_pool.tile([1, E], f32, tag="bo1")
            nc.sync.dma_start(bo_one[:], bo[:])
            bo_rep = wo_pool.tile([P, E], f32, tag="bor")
            nc.gpsimd.partition_broadcast(bo_rep[:], bo_one[:])

            wot_view = wot.rearrange("(o p) f -> p o f", p=P)
            wo_rs = []
            for nf in range(E // 512):
                wo_sb = wo_pool.tile([P, EC, 512], f32, tag="wosb")
                nc.sync.dma_start(
                    wo_sb[:], wot_view[:, :, nf * 512:(nf + 1) * 512])
                wo_r = wo_pool.tile([P, EC, 512], f32r, tag="wor")
                nc.vector.tensor_copy(wo_r[:], wo_sb[:])
                wo_rs.append(wo_r)

            on_view = on_d.rearrange("(o p) t -> p o t", p=P)
            for qc in range(QC):
                on_sb2 = wo_pool.tile([P, EC, P], f32, tag="onsb")
                nc.sync.dma_start(
                    on_sb2[:], on_view[:, :, qc * P:(qc + 1) * P])
                on_r = wo_pool.tile([P, EC, P], f32r, tag="onr")
                nc.vector.tensor_copy(on_r[:], on_sb2[:])
                for nf in range(E // 512):
                    py = ps.tile([P, 1024], f32, tag="S")
                    for hp in range(HP):
                        nc.tensor.matmul(
                            py[:, :512], on_r[:, hp, :], wo_rs[nf][:, hp, :],
                            start=(hp == 0), stop=(hp == HP - 1))
                    y_sb = yp.tile([P, 512], f32, tag="ysb")
                    nc.vector.tensor_add(
                        y_sb[:], py[:, :512],
                        bo_rep[:, nf * 512:(nf + 1) * 512])
                    nc.sync.dma_start(
                        y[qc * P:(qc + 1) * P, nf * 512:(nf + 1) * 512],
                        y_sb[:])


def _build_kernel(reps=1):
    nc = bacc.Bacc("TRN2", target_bir_lowering=False, debug=False,
                   num_devices=N_CORES)
    xt_d = nc.dram_tensor("xt", [E, S], dt.float32, kind="ExternalInput").ap()
    wqt = nc.dram_tensor("wqt", [E, E], dt.float32, kind="ExternalInput").ap()
    wkt = nc.dram_tensor("wkt", [E, E], dt.float32, kind="ExternalInput").ap()
    wvt = nc.dram_tensor("wvt", [E, E], dt.float32, kind="ExternalInput").ap()
    wot = nc.dram_tensor("wot", [E, E], dt.float32, kind="ExternalInput").ap()
    bo = nc.dram_tensor("bo", [1, E], dt.float32, kind="ExternalInput").ap()
    y = nc.dram_tensor("y", [SQ, E], dt.float32, kind="ExternalOutput").ap()

    dbg = None
    if DEBUG:
        shapes = {
            "xt": [P, EC, S], "on": [E, SQ],
        }
        dbg = {k: nc.dram_tensor(f"dbg_{k}", v, dt.float32,
                                 kind="ExternalOutput").ap()
               for k, v in shapes.items()}

    with tile.TileContext(nc) as tc:
        for _ in range(reps):
            _emit(nc, tc, xt_d, wqt, wkt, wvt, wot, bo, y, dbg)
    nc.compile()
    return nc


_NC_CACHE = None


def make_in_maps(x, Wq, Wk, Wv, Wo, bo):
    x = np.asarray(x, np.float32)
    wqt = np.ascontiguousarray(np.asarray(Wq, np.float32).T)
    wkt = np.ascontiguousarray(np.asarray(Wk, np.float32).T)
    wvt = np.ascontiguousarray(np.asarray(Wv, np.float32).T)
    wot = np.ascontiguousarray(np.asarray(Wo, np.float32).T)
    bo_ = np.ascontiguousarray(np.asarray(bo, np.float32).reshape(1, E))

    in_maps = []
    for c in range(N_CORES):
        b, half = c // 2, c % 2
        # rotate so this core's query block is rows 0:SQ (keys are a
        # permutation of the sequence -- attention is invariant to key order)
        xt_rot = np.ascontiguousarray(np.roll(x[b], -half * SQ, axis=0).T)
        in_maps.append({"xt": xt_rot, "wqt": wqt, "wkt": wkt, "wvt": wvt,
                        "wot": wot, "bo": bo_})
    return in_maps


def get_nc(reps=1):
    global _NC_CACHE
    if _NC_CACHE is None:
        _NC_CACHE = {}
    if reps not in _NC_CACHE:
        _NC_CACHE[reps] = _build_kernel(reps)
    return _NC_CACHE[reps]


def kernel(x, Wq, Wk, Wv, Wo, bo):
    nc = get_nc()
    in_maps = make_in_maps(x, Wq, Wk, Wv, Wo, bo)
    res = run_bass_kernel_spmd(nc, in_maps, core_ids=list(range(N_CORES)))
    out = np.empty((B, S, E), np.float32)
    for c in range(N_CORES):
        b, half = c // 2, c % 2
        out[b, half * SQ:(half + 1) * SQ, :] = res.results[c]["y"]
    return out



# revision 30
# speedup vs baseline: 153.3916x; 153.3916x over previous
"""Multi-head attention (B=4, S=2048, E=1024, H=16, D=64) on 8 TRN2 cores.

Sharding: core c handles batch b = c//2, query half = c%2 (1024 queries).
Each core computes K/V over its batch's full sequence, attention for all 16
heads over its 1024 queries, and the output projection for its output chunk.
Outputs are disjoint -> host gather is concatenation.

The host rotates each core's sequence so its query block is always rows
0:1024 (attention is permutation-invariant over keys), pre-transposes the
weights and activations, and casts x/W to bf16 (halves DMA, enables FWL
weight loads; error budget ~3e-3 vs the 2e-2 gate).

v2 vs v1:
- all matmul operands bf16 (except nothing: scores/proj/po/out all bf16)
- no device-side casts: DMA lands directly in matmul-ready tiles
- softmax normalize uses reciprocal_approx_fast + early PSUM evacuation so
  the PE never stalls >1us on the normalize chain (keeps HAM at K=8/8)
- ON stays SBUF-resident; out-projection of the first query half is
  interleaved into the last head-pair's attention stream
- V projection in 2 phases x 8 heads with N=512 moving operand
"""

from contextlib import ExitStack

import numpy as np

import concourse.bass as bass
import concourse.tile as tile
from concourse import bacc, mybir
from concourse.bass_utils import run_bass_kernel_spmd

dt = mybir.dt
AF = mybir.ActivationFunctionType

B, S, E, H, D = 4, 2048, 1024, 16, 64
N_CORES = 8
SQ = 1024          # queries per core
P = 128
EC = E // P        # 8 e-chunks
TC = S // P        # 16 t-chunks (keys)
QC = SQ // P       # 8 query chunks
HP = H // 2        # 8 head-pairs


def _emit(nc, tc, xt_d, wqkt, wvt, wot, bo, y):
    f32, bf16 = dt.float32, dt.bfloat16

    with ExitStack() as ctx:
        const = ctx.enter_context(tc.tile_pool(name="const", bufs=1))
        # PSUM: sc 2x[128,1024] (4 banks) + po 2x[65,512] (2) + proj 2x[128,512] (2)
        ps = ctx.enter_context(tc.tile_pool(name="ps", bufs=2, space="PSUM"))
        ps_p = ctx.enter_context(
            tc.tile_pool(name="ps_p", bufs=2, space="PSUM"))
        ps_o = ctx.enter_context(
            tc.tile_pool(name="ps_o", bufs=2, space="PSUM"))

        ones_col = const.tile([P, 1], bf16)
        nc.vector.memset(ones_col[:], 1.0)
        bo_one = const.tile([1, E], f32)
        nc.sync.dma_start(bo_one[:], bo[:])
        bo_rep = const.tile([P, E], f32)
        nc.gpsimd.partition_broadcast(bo_rep[:], bo_one[:])

        with ExitStack() as actx:
            xt_pool = actx.enter_context(tc.tile_pool(name="xt", bufs=4))
            on_pool = actx.enter_context(tc.tile_pool(name="on", bufs=1))
            w1 = actx.enter_context(tc.tile_pool(name="w1", bufs=2))
            wv_pool = actx.enter_context(tc.tile_pool(name="wv", bufs=1))
            wo_pool = actx.enter_context(tc.tile_pool(name="wo", bufs=1))
            qt_pool = actx.enter_context(tc.tile_pool(name="qt", bufs=2))
            kt_pool = actx.enter_context(tc.tile_pool(name="kt", bufs=2))
            vp_pool = actx.enter_context(tc.tile_pool(name="vp", bufs=2))
            ut_pool = actx.enter_context(tc.tile_pool(name="ut", bufs=4))
            nz = actx.enter_context(tc.tile_pool(name="nz", bufs=2))
            yp = actx.enter_context(tc.tile_pool(name="yp", bufs=2))

            # ---- xT in SBUF (bf16); queries are cols 0:1024 ----
            # host pre-tiled as [P, S/512, EC, 512]; four separate tiles so
            # consumers only wait for the chunk they read (tile-granular deps)
            xTc = [xt_pool.tile([P, EC, 512], bf16, tag="xt", name=f"xt{c}")
                   for c in range(S // 512)]
            nc.sync.dma_start(xTc[0][:], xt_d[:, 0])
            nc.sync.dma_start(xTc[1][:], xt_d[:, 1])

            # ON^T, SBUF resident: [128 rows = head-dim within pair, hp, q]
            on_all = on_pool.tile([P, HP, SQ], bf16)
            wo_all = {}

            qt_t, kt_t, vp_t = {}, {}, {}

            def proj_ops(hp):
                """Returns [(group, op)] — group labels let hp0 schedule its
                own projections against its own attention deadlines."""
                ops = []
                st = {}

                def wload():
                    st["w"] = w1.tile([P, EC, 2, P], bf16, tag="wdma",
                                      name=f"w{hp}")
                    nc.sync.dma_start(st["w"][:], wqkt[hp])
                ops.append(("w", wload))

                # QT: ec-outer with both psum banks so consecutive matmuls
                # share the stationary operand (LDW dedupe)
                def qalloc():
                    qt_t[hp] = qt_pool.tile([P, SQ], bf16, tag="qt",
                                            name=f"qt{hp}")
                    st["pq0"] = ps_p.tile([P, 512], f32, tag="PROJ",
                                          name=f"pq{hp}_0")
                    st["pq1"] = ps_p.tile([P, 512], f32, tag="PROJ",
                                          name=f"pq{hp}_1")
                ops.append(("q", qalloc))
                for ec in range(EC):
                    def qmm(ec=ec):
                        for nq in range(2):
                            nc.tensor.matmul(
                                st[f"pq{nq}"][:], st["w"][:, ec, 0],
                                xTc[nq][:, ec, :],
                                start=(ec == 0), stop=(ec == EC - 1))
                    ops.append(("q", qmm))

                def qcopy():
                    nc.vector.tensor_copy(qt_t[hp][:, 0:512], st["pq0"][:])
                    nc.vector.tensor_copy(qt_t[hp][:, 512:1024], st["pq1"][:])
                ops.append(("q", qcopy))

                # KT: four 512-chunks as two ec-outer bank pairs
                for kg in range(2):
                    def kalloc(kg=kg):
                        if kg == 0:
                            kt_t[hp] = kt_pool.tile([P, S], bf16, tag="kt",
                                                    name=f"kt{hp}")
                        st["pk0"] = ps_p.tile([P, 512], f32, tag="PROJ",
                                              name=f"pk{hp}_{kg}0")
                        st["pk1"] = ps_p.tile([P, 512], f32, tag="PROJ",
                                              name=f"pk{hp}_{kg}1")
                    ops.append((f"kg{kg}", kalloc))
                    for ec in range(EC):
                        def kmm(ec=ec, kg=kg):
                            for j in range(2):
                                nc.tensor.matmul(
                                    st[f"pk{j}"][:], st["w"][:, ec, 1],
                                    xTc[2 * kg + j][:, ec, :],
                                    start=(ec == 0), stop=(ec == EC - 1))
                        ops.append((f"kg{kg}", kmm))

                    def kcopy(kg=kg):
                        for j in range(2):
                            nk = 2 * kg + j
                            nc.vector.tensor_copy(
                                kt_t[hp][:, nk * 512:(nk + 1) * 512],
                                st[f"pk{j}"][:])
                    ops.append((f"kg{kg}", kcopy))

                # V for 8 heads (hps hp..hp+3) when hp%4==0; N=512 moving
                if hp % 4 == 0:
                    def vload(hp=hp):
                        st["wv"] = wv_pool.tile([P, EC, 512], bf16,
                                                tag="wv", name=f"wv{hp}")
                        nc.sync.dma_start(st["wv"][:], wvt[hp // 4])
                        vp_t[hp // 4] = vp_pool.tile(
                            [P, TC, 8, 65], bf16, tag="vp",
                            name=f"vp{hp // 4}")
                        nc.vector.tensor_copy(
                            vp_t[hp // 4][:, :, :, 64:65],
                            ones_col[:, None, None, :].to_broadcast(
                                [P, TC, 8, 1]))
                    ops.append(("vload", vload))
                    for tc_i in range(TC):
                        def valloc(tc_i=tc_i, hp=hp):
                            st["pv"] = ps_p.tile([P, 512], f32, tag="PROJ",
                                                 name=f"pv{hp}_{tc_i}")
                        ops.append((f"v{tc_i}", valloc))
                        for ec in range(EC):
                            def vmm(ec=ec, tc_i=tc_i):
                                nc.tensor.matmul(
                                    st["pv"][:],
                                    xTc[tc_i // 4][
                                        :, ec,
                                        (tc_i % 4) * P:(tc_i % 4 + 1) * P],
                                    st["wv"][:, ec, :],
                                    start=(ec == 0), stop=(ec == EC - 1))
                            ops.append((f"v{tc_i}", vmm))

                        def vcopy(tc_i=tc_i, hp=hp):
                            nc.vector.tensor_copy(
                                vp_t[hp // 4][:, tc_i, :, 0:64],
                                st["pv"][:].rearrange(
                                    "p (h d) -> p h d", h=8))
                        ops.append((f"v{tc_i}", vcopy))

                # prefetch Wo^T late (SBUF freed by then)
                if hp == HP - 2:
                    def woload():
                        wo_all["t"] = wo_pool.tile([P, EC, E], bf16,
                                                   tag="wo", name="wo_t")
                        nc.sync.dma_start(wo_all["t"][:], wot[:])
                    ops.append(("wo", woload))
                return ops

            def outproj_ops(qc):
                ops = []
                st = {}

                def yalloc():
                    st["py0"] = ps_p.tile([P, 512], f32, tag="PROJ",
                                          name=f"py{qc}_0")
                    st["py1"] = ps_p.tile([P, 512], f32, tag="PROJ",
                                          name=f"py{qc}_1")
                ops.append(yalloc)
                for hp in range(HP):
                    def ymm(hp=hp):
                        on_c = on_all[:, hp, qc * P:(qc + 1) * P]
                        nc.tensor.matmul(
                            st["py0"][:], on_c, wo_all["t"][:, hp, 0:512],
                            start=(hp == 0), stop=(hp == HP - 1))
                        nc.tensor.matmul(
                            st["py1"][:], on_c, wo_all["t"][:, hp, 512:1024],
                            start=(hp == 0), stop=(hp == HP - 1))
                    ops.append(ymm)

                def yout():
                    for nf, py in ((0, st["py0"]), (1, st["py1"])):
                        y_sb = yp.tile([P, 512], f32, tag="ysb")
                        nc.vector.tensor_add(
                            y_sb[:], py[:],
                            bo_rep[:, nf * 512:(nf + 1) * 512])
                        nc.sync.dma_start(
                            y[qc * P:(qc + 1) * P, nf * 512:(nf + 1) * 512],
                            y_sb[:])
                ops.append(yout)
                return ops

            # ---- prologue: just enough of hp0's projections to start its
            # attention (Q, K chunk 0-1, V tc0-1); the rest streams into
            # hp0's own kc loop under deadlines.  Weight DMAs are issued
            # before the tail xT chunk DMAs so hp0's matmuls aren't queued
            # behind them.
            ops0 = proj_ops(0)
            dma_groups = {"w", "vload"}
            pre_groups = {"q", "kg0", "v0", "v1"}
            for g, op in ops0:
                if g in dma_groups:
                    op()
            nc.sync.dma_start(xTc[2][:], xt_d[:, 2])
            nc.sync.dma_start(xTc[3][:], xt_d[:, 3])
            for g, op in ops0:
                if g in pre_groups:
                    op()
            pre_groups |= dma_groups
            # deadline schedule for hp0's leftovers, in slot units of hp0's
            # attention (slot = qh*16+kc, emitted at slot start):
            # V tc j must precede po(kc=j) -> due j-1; K group 1 must precede
            # scores(kc=8) -> due <=7 (spread 2..6).
            # NOTE: each ps_p accumulation group (kalloc..kcopy etc.) must be
            # emitted contiguously -- interleaving two groups would alias the
            # 2-buffer psum ring mid-accumulation.  kg1 goes out as one burst.
            sched0 = []
            for g, op in ops0:
                if g in pre_groups:
                    continue
                if g == "kg1":
                    sched0.append((4, op))
                else:
                    assert g.startswith("v")
                    j = int(g[1:])
                    sched0.append((j - 1, op))
            sched0.sort(key=lambda x: x[0])

            for hp in range(HP):
                qt, kt = qt_t[hp], kt_t[hp]
                vp = vp_t[hp // 4]
                ha, hb = 2 * (hp % 4), 2 * (hp % 4) + 1
                if hp == 0:
                    # own leftovers in qh0, hp1's projections in qh1
                    p1 = proj_ops(1)
                    sched = sched0 + [(16 + 16 * i // len(p1), op)
                                      for i, (g, op) in enumerate(p1)]
                elif hp + 1 < HP:
                    nxt = proj_ops(hp + 1)
                    sched = [(32 * i // len(nxt), op)
                             for i, (g, op) in enumerate(nxt)]
                else:
                    # out-proj of the first query half reads on_all[:, 7, qh0]
                    # which hp7/qh0's normalize writes -- only interleave into
                    # the qh=1 loop.
                    nxt = [op for qc in range(QC // 2)
                           for op in outproj_ops(qc)]
                    sched = [(16 + 16 * i // len(nxt), op)
                             for i, op in enumerate(nxt)]
                n_emit = 0
                it = 0

                for qh in range(2):
                    po_a = ps_o.tile([65, 512], f32, tag="po")
                    po_b = ps_o.tile([65, 512], f32, tag="po")
                    qs = slice(qh * 512, (qh + 1) * 512)
                    for kc in range(TC):
                        # emit interleaved work due at this slot
                        while n_emit < len(sched) and sched[n_emit][0] <= it:
                            sched[n_emit][1]()
                            n_emit += 1
                        sc = ps.tile([P, 1024], f32, tag="S")
                        nc.tensor.matmul(
                            sc[:, 0:512], kt[0:64, kc * P:(kc + 1) * P],
                            qt[0:64, qs], start=True, stop=True)
                        nc.tensor.matmul(
                            sc[:, 512:1024], kt[64:128, kc * P:(kc + 1) * P],
                            qt[64:128, qs], start=True, stop=True)
                        ut = ut_pool.tile([P, 1024], bf16, tag="ut")
                        nc.scalar.activation(
                            ut[:], sc[:], AF.Exp, scale=0.125)
                        nc.tensor.matmul(
                            po_a[:], vp[:, kc, ha], ut[:, 0:512],
                            start=(kc == 0), stop=(kc == TC - 1))
                        nc.tensor.matmul(
                            po_b[:], vp[:, kc, hb], ut[:, 512:1024],
                            start=(kc == 0), stop=(kc == TC - 1))
                        it += 1

                    # normalize: den copies first (start the recip chain),
                    # then evacuate po so the PE can reuse the banks.
                    den = nz.tile([1, 1024], f32, tag="den")
                    nc.vector.tensor_copy(den[:, 0:512], po_a[64:65, :])
                    nc.vector.tensor_copy(den[:, 512:1024], po_b[64:65, :])
                    unorm = nz.tile([P, 512], f32, tag="unorm")
                    nc.vector.tensor_copy(unorm[0:64, :], po_a[0:64, :])
                    nc.vector.tensor_copy(unorm[64:128, :], po_b[0:64, :])
                    rec = nz.tile([1, 1024], f32, tag="rec")
                    nc.vector.reciprocal_approx_fast(rec[:], den[:])
                    brec = nz.tile([P, 1024], f32, tag="brec")
                    nc.gpsimd.partition_broadcast(brec[:], rec[:])
                    nc.vector.tensor_mul(
                        on_all[0:64, hp, qs], unorm[0:64, :],
                        brec[0:64, 0:512])
                    nc.vector.tensor_mul(
                        on_all[64:128, hp, qs], unorm[64:128, :],
                        brec[64:128, 512:1024])
                while n_emit < len(sched):
                    sched[n_emit][1]()
                    n_emit += 1

            # tail: output projection for the second query half
            for qc in range(QC // 2, QC):
                for op in outproj_ops(qc):
                    op()


def _build_kernel(reps=1):
    nc = bacc.Bacc("TRN2", target_bir_lowering=False, debug=False,
                   num_devices=N_CORES)
    bf16 = dt.bfloat16
    # all weight/activation tensors pre-tiled on the host so every DMA is
    # one contiguous run per partition
    xt_d = nc.dram_tensor("xt", [P, S // 512, EC, 512], bf16,
                          kind="ExternalInput").ap()
    wqkt = nc.dram_tensor("wqkt", [HP, P, EC, 2, P], bf16,
                          kind="ExternalInput").ap()
    wvt = nc.dram_tensor("wvt", [2, P, EC, 512], bf16,
                         kind="ExternalInput").ap()
    wot = nc.dram_tensor("wot", [P, EC, E], bf16, kind="ExternalInput").ap()
    bo = nc.dram_tensor("bo", [1, E], dt.float32, kind="ExternalInput").ap()
    y = nc.dram_tensor("y", [SQ, E], dt.float32, kind="ExternalOutput").ap()

    with tile.TileContext(nc) as tc:
        for _ in range(reps):
            _emit(nc, tc, xt_d, wqkt, wvt, wot, bo, y)
    nc.compile()
    return nc


_NC_CACHE = None


def _bf16(a):
    import ml_dtypes
    return np.ascontiguousarray(np.asarray(a, np.float32)).astype(
        ml_dtypes.bfloat16)


def _pof(wT):
    # [E, E] row-major -> [P, EC, E] with e_row = o*128 + p
    return wT.reshape(EC, P, E).transpose(1, 0, 2)


def make_in_maps(x, Wq, Wk, Wv, Wo, bo):
    x = np.asarray(x, np.float32)
    wqp = _pof(np.asarray(Wq, np.float32).T)   # [P, EC, E]
    wkp = _pof(np.asarray(Wk, np.float32).T)
    # [HP, P, EC, 2, P]: per-head-pair contiguous chunk of (Wq|Wk)
    wqk = np.stack([wqp, wkp], axis=2)         # [P, EC, 2, E]
    wqkt = _bf16(np.ascontiguousarray(
        wqk.reshape(P, EC, 2, HP, P).transpose(3, 0, 1, 2, 4)))
    # [2, P, EC, 512]: V weights per 8-head phase
    wvp = _pof(np.asarray(Wv, np.float32).T)
    wvt = _bf16(np.ascontiguousarray(
        wvp.reshape(P, EC, 2, 512).transpose(2, 0, 1, 3)))
    wot = _bf16(np.ascontiguousarray(_pof(np.asarray(Wo, np.float32).T)))
    bo_ = np.ascontiguousarray(np.asarray(bo, np.float32).reshape(1, E))

    in_maps = []
    for c in range(N_CORES):
        b, half = c // 2, c % 2
        # rotate so this core's query block is rows 0:SQ (keys are a
        # permutation of the sequence -- attention is invariant to key order)
        xt_rot = np.roll(x[b], -half * SQ, axis=0).T    # [E, S]
        xt_p = _bf16(np.ascontiguousarray(
            xt_rot.reshape(EC, P, S // 512, 512).transpose(1, 2, 0, 3)))
        in_maps.append({"xt": xt_p, "wqkt": wqkt, "wvt": wvt,
                        "wot": wot, "bo": bo_})
    return in_maps


def get_nc(reps=1):
    global _NC_CACHE
    if _NC_CACHE is None:
        _NC_CACHE = {}
    if reps not in _NC_CACHE:
        _NC_CACHE[reps] = _build_kernel(reps)
    return _NC_CACHE[reps]


def kernel(x, Wq, Wk, Wv, Wo, bo):
    nc = get_nc()
    in_maps = make_in_maps(x, Wq, Wk, Wv, Wo, bo)
    res = run_bass_kernel_spmd(nc, in_maps, core_ids=list(range(N_CORES)))
    out = np.empty((B, S, E), np.float32)
    for c in range(N_CORES):
        b, half = c // 2, c % 2
        out[b, half * SQ:(half + 1) * SQ, :] = res.results[c]["y"]
    return out


# revision 32
# speedup vs baseline: 155.2198x; 1.0119x over previous
"""Multi-head attention (B=4, S=2048, E=1024, H=16, D=64) on 8 TRN2 cores.

Sharding: core c handles batch b = c//2, query half = c%2 (1024 queries).
Each core computes K/V over its batch's full sequence, attention for all 16
heads over its 1024 queries, and the output projection for its output chunk.
Outputs are disjoint -> host gather is concatenation.

The host rotates each core's sequence so its query block is always rows
0:1024 (attention is permutation-invariant over keys), pre-transposes the
weights and activations, and casts x/W to bf16 (halves DMA, enables FWL
weight loads; error budget ~3e-3 vs the 2e-2 gate).

v2 vs v1:
- all matmul operands bf16 (except nothing: scores/proj/po/out all bf16)
- no device-side casts: DMA lands directly in matmul-ready tiles
- softmax normalize uses reciprocal_approx_fast + early PSUM evacuation so
  the PE never stalls >1us on the normalize chain (keeps HAM at K=8/8)
- ON stays SBUF-resident; out-projection of the first query half is
  interleaved into the last head-pair's attention stream
- V projection in 2 phases x 8 heads with N=512 moving operand
"""

from contextlib import ExitStack

import numpy as np

import concourse.bass as bass
import concourse.tile as tile
from concourse import bacc, mybir
from concourse.bass_utils import run_bass_kernel_spmd

dt = mybir.dt
AF = mybir.ActivationFunctionType

B, S, E, H, D = 4, 2048, 1024, 16, 64
N_CORES = 8
SQ = 1024          # queries per core
P = 128
EC = E // P        # 8 e-chunks
TC = S // P        # 16 t-chunks (keys)
QC = SQ // P       # 8 query chunks
HP = H // 2        # 8 head-pairs


def _emit(nc, tc, xt_d, wqkt, wvt, wot, bo, y):
    f32, bf16 = dt.float32, dt.bfloat16

    with ExitStack() as ctx:
        const = ctx.enter_context(tc.tile_pool(name="const", bufs=1))
        # PSUM: sc 2x[128,1024] (4 banks) + po 2x[65,512] (2) + proj 2x[128,512] (2)
        ps = ctx.enter_context(tc.tile_pool(name="ps", bufs=2, space="PSUM"))
        ps_p = ctx.enter_context(
            tc.tile_pool(name="ps_p", bufs=2, space="PSUM"))
        ps_o = ctx.enter_context(
            tc.tile_pool(name="ps_o", bufs=2, space="PSUM"))

        ones_col = const.tile([P, 1], bf16)
        nc.vector.memset(ones_col[:], 1.0)
        bo_one = const.tile([1, E], f32)
        nc.sync.dma_start(bo_one[:], bo[:])
        bo_rep = const.tile([P, E], f32)
        nc.gpsimd.partition_broadcast(bo_rep[:], bo_one[:])

        # PE warm-up: dummy matmuls during the DMA head keep the HAM clock
        # gate busy so it reaches 8/8 before the real matmul stream starts
        # (cold matmuls run at 1.2 instead of 2.4 GHz).
        wu = const.tile([P, 512], dt.bfloat16)
        nc.vector.memset(wu[:], 0.0)
        pwu = ps_p.tile([1, 512], f32, tag="PROJ", name="pwu")
        for _ in range(12):
            nc.tensor.matmul(pwu[:], ones_col[:], wu[:],
                             start=True, stop=True)

        with ExitStack() as actx:
            xt_pool = actx.enter_context(tc.tile_pool(name="xt", bufs=4))
            on_pool = actx.enter_context(tc.tile_pool(name="on", bufs=1))
            w1 = actx.enter_context(tc.tile_pool(name="w1", bufs=2))
            wv_pool = actx.enter_context(tc.tile_pool(name="wv", bufs=1))
            wo_pool = actx.enter_context(tc.tile_pool(name="wo", bufs=1))
            qt_pool = actx.enter_context(tc.tile_pool(name="qt", bufs=2))
            kt_pool = actx.enter_context(tc.tile_pool(name="kt", bufs=2))
            vp_pool = actx.enter_context(tc.tile_pool(name="vp", bufs=2))
            ut_pool = actx.enter_context(tc.tile_pool(name="ut", bufs=4))
            nz = actx.enter_context(tc.tile_pool(name="nz", bufs=2))
            yp = actx.enter_context(tc.tile_pool(name="yp", bufs=2))

            # ---- xT in SBUF (bf16); queries are cols 0:1024 ----
            # host pre-tiled as [P, S/512, EC, 512]; four separate tiles so
            # consumers only wait for the chunk they read (tile-granular deps)
            xTc = [xt_pool.tile([P, EC, 512], bf16, tag="xt", name=f"xt{c}")
                   for c in range(S // 512)]

            # ON^T, SBUF resident: [128 rows = head-dim within pair, hp, q]
            on_all = on_pool.tile([P, HP, SQ], bf16)
            wo_all = {}

            qt_t, kt_t, vp_t = {}, {}, {}

            def proj_ops(hp):
                """Returns [(group, op)] — group labels let hp0 schedule its
                own projections against its own attention deadlines."""
                ops = []
                st = {}

                def wload():
                    st["w"] = w1.tile([P, EC, 2, P], bf16, tag="wdma",
                                      name=f"w{hp}")
                    nc.sync.dma_start(st["w"][:], wqkt[hp])
                ops.append(("w", wload))

                # QT: ec-outer with both psum banks so consecutive matmuls
                # share the stationary operand (LDW dedupe)
                def qalloc():
                    qt_t[hp] = qt_pool.tile([P, SQ], bf16, tag="qt",
                                            name=f"qt{hp}")
                    st["pq0"] = ps_p.tile([P, 512], f32, tag="PROJ",
                                          name=f"pq{hp}_0")
                    st["pq1"] = ps_p.tile([P, 512], f32, tag="PROJ",
                                          name=f"pq{hp}_1")
                ops.append(("q", qalloc))
                for ec in range(EC):
                    def qmm(ec=ec):
                        for nq in range(2):
                            nc.tensor.matmul(
                                st[f"pq{nq}"][:], st["w"][:, ec, 0],
                                xTc[nq][:, ec, :],
                                start=(ec == 0), stop=(ec == EC - 1))
                    ops.append(("q", qmm))

                def qcopy():
                    nc.vector.tensor_copy(qt_t[hp][:, 0:512], st["pq0"][:])
                    nc.vector.tensor_copy(qt_t[hp][:, 512:1024], st["pq1"][:])
                ops.append(("q", qcopy))

                # KT: four 512-chunks as two ec-outer bank pairs
                for kg in range(2):
                    def kalloc(kg=kg):
                        if kg == 0:
                            kt_t[hp] = kt_pool.tile([P, S], bf16, tag="kt",
                                                    name=f"kt{hp}")
                        st["pk0"] = ps_p.tile([P, 512], f32, tag="PROJ",
                                              name=f"pk{hp}_{kg}0")
                        st["pk1"] = ps_p.tile([P, 512], f32, tag="PROJ",
                                              name=f"pk{hp}_{kg}1")
                    ops.append((f"kg{kg}", kalloc))
                    for ec in range(EC):
                        def kmm(ec=ec, kg=kg):
                            for j in range(2):
                                nc.tensor.matmul(
                                    st[f"pk{j}"][:], st["w"][:, ec, 1],
                                    xTc[2 * kg + j][:, ec, :],
                                    start=(ec == 0), stop=(ec == EC - 1))
                        ops.append((f"kg{kg}", kmm))

                    def kcopy(kg=kg):
                        for j in range(2):
                            nk = 2 * kg + j
                            nc.vector.tensor_copy(
                                kt_t[hp][:, nk * 512:(nk + 1) * 512],
                                st[f"pk{j}"][:])
                    ops.append((f"kg{kg}", kcopy))

                # V for 8 heads (hps hp..hp+3) when hp%4==0; N=512 moving
                if hp % 4 == 0:
                    def vload(hp=hp):
                        st["wv"] = wv_pool.tile([P, EC, 512], bf16,
                                                tag="wv", name=f"wv{hp}")
                        nc.sync.dma_start(st["wv"][:], wvt[hp // 4])
                        vp_t[hp // 4] = vp_pool.tile(
                            [P, TC, 8, 65], bf16, tag="vp",
                            name=f"vp{hp // 4}")
                        nc.vector.tensor_copy(
                            vp_t[hp // 4][:, :, :, 64:65],
                            ones_col[:, None, None, :].to_broadcast(
                                [P, TC, 8, 1]))
                    ops.append(("vload", vload))
                    for tc_i in range(TC):
                        def valloc(tc_i=tc_i, hp=hp):
                            st["pv"] = ps_p.tile([P, 512], f32, tag="PROJ",
                                                 name=f"pv{hp}_{tc_i}")
                        ops.append((f"v{tc_i}", valloc))
                        for ec in range(EC):
                            def vmm(ec=ec, tc_i=tc_i):
                                nc.tensor.matmul(
                                    st["pv"][:],
                                    xTc[tc_i // 4][
                                        :, ec,
                                        (tc_i % 4) * P:(tc_i % 4 + 1) * P],
                                    st["wv"][:, ec, :],
                                    start=(ec == 0), stop=(ec == EC - 1))
                            ops.append((f"v{tc_i}", vmm))

                        def vcopy(tc_i=tc_i, hp=hp):
                            nc.vector.tensor_copy(
                                vp_t[hp // 4][:, tc_i, :, 0:64],
                                st["pv"][:].rearrange(
                                    "p (h d) -> p h d", h=8))
                        ops.append((f"v{tc_i}", vcopy))

                # prefetch Wo^T late (SBUF freed by then)
                if hp == HP - 2:
                    def woload():
                        wo_all["t"] = wo_pool.tile([P, EC, E], bf16,
                                                   tag="wo", name="wo_t")
                        nc.sync.dma_start(wo_all["t"][:], wot[:])
                    ops.append(("wo", woload))
                return ops

            def outproj_ops(qc, pool=None, tag="PROJ"):
                pool = ps_p if pool is None else pool
                ops = []
                st = {}

                def yalloc():
                    st["py0"] = pool.tile([P, 512], f32, tag=tag,
                                          name=f"py{qc}_0")
                    st["py1"] = pool.tile([P, 512], f32, tag=tag,
                                          name=f"py{qc}_1")
                ops.append(yalloc)
                for hp in range(HP):
                    def ymm(hp=hp):
                        on_c = on_all[:, hp, qc * P:(qc + 1) * P]
                        nc.tensor.matmul(
                            st["py0"][:], on_c, wo_all["t"][:, hp, 0:512],
                            start=(hp == 0), stop=(hp == HP - 1))
                        nc.tensor.matmul(
                            st["py1"][:], on_c, wo_all["t"][:, hp, 512:1024],
                            start=(hp == 0), stop=(hp == HP - 1))
                    ops.append(ymm)

                def yout():
                    for nf, py in ((0, st["py0"]), (1, st["py1"])):
                        y_sb = yp.tile([P, 512], f32, tag="ysb")
                        nc.vector.tensor_add(
                            y_sb[:], py[:],
                            bo_rep[:, nf * 512:(nf + 1) * 512])
                        nc.sync.dma_start(
                            y[qc * P:(qc + 1) * P, nf * 512:(nf + 1) * 512],
                            y_sb[:])
                ops.append(yout)
                return ops

            # ---- prologue: just enough of hp0's projections to start its
            # attention (Q, K chunk 0-1, V tc0-1); the rest streams into
            # hp0's own kc loop under deadlines.  Weight DMAs are issued
            # before the tail xT chunk DMAs so hp0's matmuls aren't queued
            # behind them.
            ops0 = proj_ops(0)
            dma_groups = {"w", "vload"}
            pre_groups = {"q", "kg0", "v0", "v1"}
            for g, op in ops0:
                if g in dma_groups:
                    op()
            nc.sync.dma_start(xTc[0][:], xt_d[:, 0])
            nc.sync.dma_start(xTc[1][:], xt_d[:, 1])
            nc.sync.dma_start(xTc[2][:], xt_d[:, 2])
            nc.sync.dma_start(xTc[3][:], xt_d[:, 3])
            for g, op in ops0:
                if g in pre_groups:
                    op()
            pre_groups |= dma_groups
            # deadline schedule for hp0's leftovers, in slot units of hp0's
            # attention (slot = qh*16+kc, emitted at slot start):
            # V tc j must precede po(kc=j) -> due j-1; K group 1 must precede
            # scores(kc=8) -> due <=7 (spread 2..6).
            # NOTE: each ps_p accumulation group (kalloc..kcopy etc.) must be
            # emitted contiguously -- interleaving two groups would alias the
            # 2-buffer psum ring mid-accumulation.  kg1 goes out as one burst.
            sched0 = []
            for g, op in ops0:
                if g in pre_groups:
                    continue
                if g == "kg1":
                    sched0.append((4, op))
                else:
                    assert g.startswith("v")
                    j = int(g[1:])
                    sched0.append((j - 1, op))
            sched0.sort(key=lambda x: x[0])

            for hp in range(HP):
                qt, kt = qt_t[hp], kt_t[hp]
                vp = vp_t[hp // 4]
                ha, hb = 2 * (hp % 4), 2 * (hp % 4) + 1
                if hp == 0:
                    # own leftovers in qh0, hp1's projections in qh1
                    p1 = proj_ops(1)
                    sched = sched0 + [(16 + 16 * i // len(p1), op)
                                      for i, (g, op) in enumerate(p1)]
                elif hp + 1 < HP:
                    nxt = proj_ops(hp + 1)
                    sched = [(32 * i // len(nxt), op)
                             for i, (g, op) in enumerate(nxt)]
                else:
                    # out-proj of the first query half reads on_all[:, 7, qh0]
                    # which hp7/qh0's normalize writes -- only interleave into
                    # the qh=1 loop.
                    nxt = [op for qc in range(QC // 2)
                           for op in outproj_ops(qc)]
                    sched = [(16 + 16 * i // len(nxt), op)
                             for i, op in enumerate(nxt)]
                n_emit = 0
                it = 0

                for qh in range(2):
                    po_a = ps_o.tile([65, 512], f32, tag="po")
                    po_b = ps_o.tile([65, 512], f32, tag="po")
                    qs = slice(qh * 512, (qh + 1) * 512)
                    for kc in range(TC):
                        # emit interleaved work due at this slot
                        while n_emit < len(sched) and sched[n_emit][0] <= it:
                            sched[n_emit][1]()
                            n_emit += 1
                        sc = ps.tile([P, 1024], f32, tag="S")
                        nc.tensor.matmul(
                            sc[:, 0:512], kt[0:64, kc * P:(kc + 1) * P],
                            qt[0:64, qs], start=True, stop=True)
                        nc.tensor.matmul(
                            sc[:, 512:1024], kt[64:128, kc * P:(kc + 1) * P],
                            qt[64:128, qs], start=True, stop=True)
                        ut = ut_pool.tile([P, 1024], bf16, tag="ut")
                        nc.scalar.activation(
                            ut[:], sc[:], AF.Exp, scale=0.125)
                        nc.tensor.matmul(
                            po_a[:], vp[:, kc, ha], ut[:, 0:512],
                            start=(kc == 0), stop=(kc == TC - 1))
                        nc.tensor.matmul(
                            po_b[:], vp[:, kc, hb], ut[:, 512:1024],
                            start=(kc == 0), stop=(kc == TC - 1))
                        it += 1

                    # normalize: den copies first (start the recip chain),
                    # then evacuate po so the PE can reuse the banks.
                    den = nz.tile([1, 1024], f32, tag="den")
                    nc.vector.tensor_copy(den[:, 0:512], po_a[64:65, :])
                    nc.vector.tensor_copy(den[:, 512:1024], po_b[64:65, :])
                    unorm = nz.tile([P, 512], f32, tag="unorm")
                    nc.vector.tensor_copy(unorm[0:64, :], po_a[0:64, :])
                    nc.vector.tensor_copy(unorm[64:128, :], po_b[0:64, :])
                    rec = nz.tile([1, 1024], f32, tag="rec")
                    nc.vector.reciprocal_approx_fast(rec[:], den[:])
                    brec = nz.tile([P, 1024], f32, tag="brec")
                    nc.gpsimd.partition_broadcast(brec[:], rec[:])
                    nc.vector.tensor_mul(
                        on_all[0:64, hp, qs], unorm[0:64, :],
                        brec[0:64, 0:512])
                    nc.vector.tensor_mul(
                        on_all[64:128, hp, qs], unorm[64:128, :],
                        brec[64:128, 512:1024])
                while n_emit < len(sched):
                    sched[n_emit][1]()
                    n_emit += 1

            # tail: output projection for the second query half; spread
            # accumulators across the attention-idle psum pools so the four
            # q-chunks pipeline instead of serializing on the proj ring
            tail_pools = [(ps, "S"), (ps_o, "po"), (ps_p, "PROJ"),
                          (ps, "S")]
            for qc, (pool, tag) in zip(range(QC // 2, QC), tail_pools):
                for op in outproj_ops(qc, pool, tag):
                    op()


def _build_kernel(reps=1):
    nc = bacc.Bacc("TRN2", target_bir_lowering=False, debug=False,
                   num_devices=N_CORES)
    bf16 = dt.bfloat16
    # all weight/activation tensors pre-tiled on the host so every DMA is
    # one contiguous run per partition
    xt_d = nc.dram_tensor("xt", [P, S // 512, EC, 512], bf16,
                          kind="ExternalInput").ap()
    wqkt = nc.dram_tensor("wqkt", [HP, P, EC, 2, P], bf16,
                          kind="ExternalInput").ap()
    wvt = nc.dram_tensor("wvt", [2, P, EC, 512], bf16,
                         kind="ExternalInput").ap()
    wot = nc.dram_tensor("wot", [P, EC, E], bf16, kind="ExternalInput").ap()
    bo = nc.dram_tensor("bo", [1, E], dt.float32, kind="ExternalInput").ap()
    y = nc.dram_tensor("y", [SQ, E], dt.float32, kind="ExternalOutput").ap()

    with tile.TileContext(nc) as tc:
        for _ in range(reps):
            _emit(nc, tc, xt_d, wqkt, wvt, wot, bo, y)
    nc.compile()
    return nc


_NC_CACHE = None


def _bf16(a):
    import ml_dtypes
    return np.ascontiguousarray(np.asarray(a, np.float32)).astype(
        ml_dtypes.bfloat16)


def _pof(wT):
    # [E, E] row-major -> [P, EC, E] with e_row = o*128 + p
    return wT.reshape(EC, P, E).transpose(1, 0, 2)


def make_in_maps(x, Wq, Wk, Wv, Wo, bo):
    x = np.asarray(x, np.float32)
    wqp = _pof(np.asarray(Wq, np.float32).T)   # [P, EC, E]
    wkp = _pof(np.asarray(Wk, np.float32).T)
    # [HP, P, EC, 2, P]: per-head-pair contiguous chunk of (Wq|Wk)
    wqk = np.stack([wqp, wkp], axis=2)         # [P, EC, 2, E]
    wqkt = _bf16(np.ascontiguousarray(
        wqk.reshape(P, EC, 2, HP, P).transpose(3, 0, 1, 2, 4)))
    # [2, P, EC, 512]: V weights per 8-head phase
    wvp = _pof(np.asarray(Wv, np.float32).T)
    wvt = _bf16(np.ascontiguousarray(
        wvp.reshape(P, EC, 2, 512).transpose(2, 0, 1, 3)))
    wot = _bf16(np.ascontiguousarray(_pof(np.asarray(Wo, np.float32).T)))
    bo_ = np.ascontiguousarray(np.asarray(bo, np.float32).reshape(1, E))

    in_maps = []
    for c in range(N_CORES):
        b, half = c // 2, c % 2
        # rotate so this core's query block is rows 0:SQ (keys are a
        # permutation of the sequence -- attention is invariant to key order)
        xt_rot = np.roll(x[b], -half * SQ, axis=0).T    # [E, S]
        xt_p = _bf16(np.ascontiguousarray(
            xt_rot.reshape(EC, P, S // 512, 512).transpose(1, 2, 0, 3)))
        in_maps.append({"xt": xt_p, "wqkt": wqkt, "wvt": wvt,
                        "wot": wot, "bo": bo_})
    return in_maps


def get_nc(reps=1):
    global _NC_CACHE
    if _NC_CACHE is None:
        _NC_CACHE = {}
    if reps not in _NC_CACHE:
        _NC_CACHE[reps] = _build_kernel(reps)
    return _NC_CACHE[reps]


def kernel(x, Wq, Wk, Wv, Wo, bo):
    nc = get_nc()
    in_maps = make_in_maps(x, Wq, Wk, Wv, Wo, bo)
    res = run_bass_kernel_spmd(nc, in_maps, core_ids=list(range(N_CORES)))
    out = np.empty((B, S, E), np.float32)
    for c in range(N_CORES):
        b, half = c // 2, c % 2
        out[b, half * SQ:(half + 1) * SQ, :] = res.results[c]["y"]
    return out
